# revision 21
# baseline (speedup 1.0000x reference)
"""Trainium2 Bass kernel for DPL safe-policy head.

Computes, for x:[B,H] and three tiny heads Wg/Wp/Wa (4/4/5 logits):
    ghost  = softmax(x@Wg + bg); pacman = softmax(x@Wp + bp); base = softmax(x@Wa + ba)
    unsafe[b,a] = sum_cd pacman[b,c] * T[a,c,d] * ghost[b,d]   (T fixed 0/1 tensor)
    out = base*(1-unsafe) / sum(...)

Closed form used on device (softmax normalizations cancel except ghost/pacman's,
which fold into Sp*Sg):
    E = exp(logits), Sg = sum(EG), Sp = sum(EP), SS = Sp*Sg
    u0 = sum_c EPc*EGc ; u1 = EP0*EG1+EP2*EG3 ; u2 = EP1*EG0+EP3*EG2
    t_j = EA_j * (SS - u_j)  (u3 = u4 = 0);  out_j = t_j / sum_j t_j

Sharding: pure data parallel over batch across 8 cores (2048 rows each).

The kernel is memory-bound on the x stream, so the host pre-packs x into
the exact operand the PE needs: per batch tile t and contraction chunk c,
the fp16 block xt[t][hh][c][bb] = x[t*128+bb, c*128+hh]. That is:
  - fp16 halves the HBM traffic (8.4MB/core instead of 16.7MB fp32); the
    fp16 rounding is identical to the on-device ACT convert the previous
    version used (max rel err 1.46e-3 vs fp32 reference, gate 2e-2).
  - the transpose happens on the host, so the device pipeline loses the
    ACT convert pass (2us/tile, was the #2 engine), the PE transpose pass
    (107ns/chunk in situ), and the DVE PSUM->SBUF copy stream (22us) -
    each PE accum matmul LDWs its [128,128] chunk straight from the
    streamed tile (256B contiguous lines -> fast weight load).

Per core pipeline:
  - xt streams through the sync HWDGE queue as [128, 16*128] fp16 tiles
    (4 KiB lines, ~420GB/s with all 16 SDMA engines); tiles 0/15 are
    quartered to shorten the startup ramp and the tail.
  - constants (w pre-transposed [128, 16*13] fp16 + bias) ride the scalar
    HWDGE ring (never the gpsimd SWDGE: its descriptor rings contend with
    SDMA engines 7/15 and straggle the stream).
  - PE: per chunk one fp16 matmul, N=13, accumulated over the 16 chunks
    of a tile in PSUM (LDW ~35-90ns/chunk; far under the stream pace).
  - DVE: per-tile bias-add fold (PSUM read) + the logic-layer tail per
    half-batch; the mid-kernel half puts elementwise products on gpsimd,
    the final latency-exposed half runs entirely on DVE.
  - outputs: one contiguous [128, 40] block per half, emitted after the
    main loop on the sync ring (they drain behind the x backlog; nothing
    downstream reads them).

History: 95.1us (f16x3 on-device-transpose baseline) -> ~75us (f16x1 +
engine re-assignment + scheduling) -> host-pretransposed fp16 stream.
"""

import numpy as np

import concourse.bacc as bacc
import concourse.mybir as mybir
import concourse.tile as tile
from concourse.bass_utils import run_bass_kernel_spmd

F32 = mybir.dt.float32
F16 = mybir.dt.float16
AX = mybir.AxisListType
ADD = mybir.AluOpType.add

MODE = "f16host"

N_CORES = 8
B_FULL, H = 16384, 2048
B = B_FULL // N_CORES  # rows per core
P = 128
NT = B // P            # batch tiles per core
NCH = H // P           # contraction chunks
J = 13                 # 4 + 4 + 5 logits
NTAILS = 2
NTQ = NT // NTAILS     # tiles per tail half


def _build_program(mode):
    assert mode == "f16host"
    nc = bacc.Bacc("TRN2", target_bir_lowering=False, debug=False,
                   num_devices=N_CORES)
    # xt[pair][hh][u][c][bb] = x[(2*pair+u)*128+bb, c*128+hh] in fp16
    # (host pre-packed): two batch tiles share a DRAM row, so the bulk
    # stream moves 8 KiB partition lines in 1 MiB transfers.
    xt_d = nc.dram_tensor("xt", [NT // 2, P, 2 * NCH * P], F16,
                          kind="ExternalInput")
    w_d = nc.dram_tensor("w", [P, NCH * J], F16, kind="ExternalInput")
    b_d = nc.dram_tensor("b", [P, J], F32, kind="ExternalInput")
    y_d = nc.dram_tensor("y", [P, NT * 5], F32, kind="ExternalOutput")

    with tile.TileContext(nc) as tc:
        with (
            tc.tile_pool(name="const", bufs=1) as cpool,
            tc.tile_pool(name="xts", bufs=1) as xts_pool,
            tc.tile_pool(name="xtsq", bufs=1) as xtsq_pool,
            tc.tile_pool(name="acc", bufs=4, space="PSUM") as acc_pool,
            tc.tile_pool(name="work", bufs=1) as wpool,
            tc.tile_pool(name="tailp", bufs=2) as tpool,
        ):
            w_sb = cpool.tile([P, NCH, J], F16)
            b_sb = cpool.tile([P, J], F32)
            # constants first on the scalar ring: issued ~7.2us, land ~9us,
            # first accum needs w ~10.5us
            nc.scalar.dma_start(
                w_sb[:].rearrange("p c j -> p (c j)"), w_d.ap())
            nc.scalar.dma_start(b_sb[:], b_d.ap())

            all_st = [wpool.tile([P, NTQ, J], F32, tag=f"st{g}",
                                 name=f"all_st{g}")
                      for g in range(NTAILS)]
            out_tiles = [wpool.tile([P, NTQ, 5], F32, tag=f"ot{g}",
                                    name=f"out_sb{g}")
                         for g in range(NTAILS)]
            out_dmas = []

            def tail(g):
                # half 0 runs mid-kernel: elementwise products on the idle
                # gpsimd so DVE stays light (free-dim reductions +
                # reciprocal must stay on DVE). The final half is
                # latency-exposed: all-DVE avoids cross-engine hops.
                ew = nc.vector if g == NTAILS - 1 else nc.gpsimd
                st = all_st[g][:]
                e_all = tpool.tile([P, NTQ, J], F32, tag="e_all")
                nc.scalar.activation(e_all[:], st,
                                     mybir.ActivationFunctionType.Exp)
                EG = e_all[:, :, 0:4]
                EP = e_all[:, :, 4:8]
                EA = e_all[:, :, 8:13]

                tmp4 = tpool.tile([P, NTQ, 4], F32, tag="tmp4")
                ew.tensor_mul(tmp4[:], EP, EG)
                tmp2 = tpool.tile([P, NTQ, 2], F32, tag="tmp2")
                ew.tensor_mul(tmp2[:], e_all[:, :, 4:8:2],
                              e_all[:, :, 1:4:2])
                tmp2b = tpool.tile([P, NTQ, 2], F32, tag="tmp2b")
                ew.tensor_mul(tmp2b[:], e_all[:, :, 5:8:2],
                              e_all[:, :, 0:3:2])

                sg = tpool.tile([P, NTQ], F32, tag="sg")
                nc.vector.tensor_reduce(sg[:], EG, axis=AX.X, op=ADD)
                sp = tpool.tile([P, NTQ], F32, tag="sp")
                nc.vector.tensor_reduce(sp[:], EP, axis=AX.X, op=ADD)
                u3 = tpool.tile([P, NTQ, 3], F32, tag="u3")
                nc.vector.tensor_reduce(u3[:, :, 0], tmp4[:], axis=AX.X,
                                        op=ADD)
                nc.vector.tensor_reduce(u3[:, :, 1], tmp2[:], axis=AX.X,
                                        op=ADD)
                nc.vector.tensor_reduce(u3[:, :, 2], tmp2b[:], axis=AX.X,
                                        op=ADD)

                ss = tpool.tile([P, NTQ], F32, tag="ss")
                ew.tensor_mul(ss[:], sp[:], sg[:])
                V = tpool.tile([P, NTQ, 5], F32, tag="V")
                ew.tensor_sub(V[:, :, 0:3],
                              ss[:].broadcast_to([P, NTQ, 3]), u3[:])
                ew.tensor_copy(V[:, :, 3:5],
                               ss[:].broadcast_to([P, NTQ, 2]))
                tj = tpool.tile([P, NTQ, 5], F32, tag="tj")
                ew.tensor_mul(tj[:], EA, V[:])

                s5 = tpool.tile([P, NTQ], F32, tag="s5")
                nc.vector.tensor_reduce(s5[:], tj[:], axis=AX.X, op=ADD)
                r5 = tpool.tile([P, NTQ], F32, tag="r5")
                nc.vector.reciprocal(r5[:], s5[:])

                out_sb = out_tiles[g]
                ew.tensor_mul(out_sb[:], tj[:],
                              r5[:].broadcast_to([P, NTQ, 5]))
                out_dmas.append((g, out_sb))

            def fold(t, acc):
                # PSUM->SBUF logits stage with the bias add fused, on DVE
                nc.vector.tensor_add(all_st[t // NTQ][:, t % NTQ, :],
                                     acc[:], b_sb[:])
                # defer the mid-kernel tail a couple of tiles so its sems
                # never gate anything on the still-busy engines
                if t == NTQ + 1:
                    tail(0)
                elif t == NT - 1:
                    tail(1)

            # Stream plan: pairs 0..6 as full 1 MiB transfers (8 KiB
            # lines, fewest issues -> deepest queue during the ramp);
            # tile 14 as a single, tile 15 as quarters so the tail chain
            # starts as soon as the last 128 KiB lands. All tiles stay
            # SBUF-resident (no pool-recycle gating of the stream).
            CP = NCH * P                  # xt columns per tile
            tile_src = {}

            # single ring for the whole stream: a dual-ring split was
            # measured slower (the rings' packets interleave per SDMA
            # engine and break HBM access locality).
            for pr in range(NT // 2 - 1):
                xpt = xts_pool.tile([P, 2 * CP], F16, tag=f"pr{pr}",
                                    name=f"pr{pr}")
                nc.sync.dma_start(xpt[:], xt_d.ap()[pr, :, :])
                tile_src[2 * pr] = (xpt, 0)
                tile_src[2 * pr + 1] = (xpt, NCH)
            x14 = xts_pool.tile([P, CP], F16, tag="x14", name="x14")
            nc.sync.dma_start(x14[:], xt_d.ap()[NT // 2 - 1, :, 0:CP])
            tile_src[NT - 2] = (x14, 0)
            for q in range(4):
                CW = CP // 4
                xq15 = xtsq_pool.tile([P, CW], F16, tag=f"q{q}",
                                      name=f"x15_{q}")
                nc.sync.dma_start(
                    xq15[:],
                    xt_d.ap()[NT // 2 - 1, :, CP + q * CW:CP + (q + 1) * CW])
                tile_src[(NT - 1, q)] = (xq15, 0)

            for t in range(NT):
                acc = acc_pool.tile([P, J], F32)
                for c in range(NCH):
                    if t == NT - 1:
                        xq, coff = tile_src[(t, c // (NCH // 4))]
                        k = c % (NCH // 4)
                    else:
                        xq, coff = tile_src[t]
                        k = coff + c
                    nc.tensor.matmul(acc[:], xq[:, k * P:(k + 1) * P],
                                     w_sb[:, c, :],
                                     start=c == 0, stop=c == NCH - 1,
                                     skip_group_check=True)
                fold(t, acc)

            # output DMAs last: the sync engine has finished its x issues,
            # and the ring drains them after the x backlog.
            for g, out_sb in out_dmas:
                nc.sync.dma_start(
                    y_d.ap()[:, g * NTQ * 5:(g + 1) * NTQ * 5],
                    out_sb[:].rearrange("p t j -> p (t j)"))

    nc.compile()
    return nc


_NC_CACHE = {}


def _get_program(mode=MODE):
    if mode not in _NC_CACHE:
        _NC_CACHE[mode] = _build_program(mode)
    return _NC_CACHE[mode]


def _prep_in_maps(x, Wg, bg, Wp, bp, Wa, ba, mode=MODE):
    x = np.asarray(x, dtype=np.float32)
    W = np.concatenate([np.asarray(Wg), np.asarray(Wp), np.asarray(Wa)],
                       axis=1).astype(np.float32)
    bvec = np.concatenate([np.asarray(bg), np.asarray(bp), np.asarray(ba)]
                          ).astype(np.float32).reshape(1, J)
    # [h, j] -> [p, c*J+j] with h = c*128 + p (contiguous device load)
    w_dev = np.ascontiguousarray(
        W.astype(np.float16).reshape(NCH, P, J).transpose(1, 0, 2)
    ).reshape(P, NCH * J)
    b_dev = np.ascontiguousarray(np.broadcast_to(bvec, (P, J)),
                                 dtype=np.float32)
    # host pre-pack: xt[pair][hh][u][c][bb] = x[(2*pair+u)*128+bb, c*128+hh]
    x16 = x.astype(np.float16)
    in_maps = []
    for i in range(N_CORES):
        xc = x16[i * B:(i + 1) * B].reshape(NT // 2, 2, P, NCH, P)
        xt = np.ascontiguousarray(xc.transpose(0, 4, 1, 3, 2)).reshape(
            NT // 2, P, 2 * NCH * P)
        in_maps.append({
            "xt": xt,
            "w": w_dev,
            "b": b_dev,
        })
    return in_maps


def kernel(x, Wg, bg, Wp, bp, Wa, ba):
    in_maps = _prep_in_maps(x, Wg, bg, Wp, bp, Wa, ba)
    nc = _get_program()
    res = run_bass_kernel_spmd(nc, in_maps, core_ids=list(range(N_CORES)))
    outs = []
    for i in range(N_CORES):
        y = res.results[i]["y"]  # [P, NT*5], row b = t*P + p at [p, t*5+j]
        outs.append(y.reshape(P, NT, 5).transpose(1, 0, 2).reshape(B, 5))
    return np.concatenate(outs, axis=0)


# revision 22
# speedup vs baseline: 1.0140x; 1.0140x over previous
"""Trainium2 Bass kernel for DPL safe-policy head.

Computes, for x:[B,H] and three tiny heads Wg/Wp/Wa (4/4/5 logits):
    ghost  = softmax(x@Wg + bg); pacman = softmax(x@Wp + bp); base = softmax(x@Wa + ba)
    unsafe[b,a] = sum_cd pacman[b,c] * T[a,c,d] * ghost[b,d]   (T fixed 0/1 tensor)
    out = base*(1-unsafe) / sum(...)

Closed form used on device (softmax normalizations cancel except ghost/pacman's,
which fold into Sp*Sg):
    E = exp(logits), Sg = sum(EG), Sp = sum(EP), SS = Sp*Sg
    u0 = sum_c EPc*EGc ; u1 = EP0*EG1+EP2*EG3 ; u2 = EP1*EG0+EP3*EG2
    t_j = EA_j * (SS - u_j)  (u3 = u4 = 0);  out_j = t_j / sum_j t_j

Sharding: pure data parallel over batch across 8 cores (2048 rows each).

The kernel is memory-bound on the x stream, so the host pre-packs x into
the exact operand the PE needs: per batch tile t and contraction chunk c,
the fp16 block xt[t][hh][c][bb] = x[t*128+bb, c*128+hh]. That is:
  - fp16 halves the HBM traffic (8.4MB/core instead of 16.7MB fp32); the
    fp16 rounding is identical to the on-device ACT convert the previous
    version used (max rel err 1.46e-3 vs fp32 reference, gate 2e-2).
  - the transpose happens on the host, so the device pipeline loses the
    ACT convert pass (2us/tile, was the #2 engine), the PE transpose pass
    (107ns/chunk in situ), and the DVE PSUM->SBUF copy stream (22us) -
    each PE accum matmul LDWs its [128,128] chunk straight from the
    streamed tile (256B contiguous lines -> fast weight load).

Per core pipeline:
  - xt streams through the sync HWDGE queue, two batch tiles per 1 MiB
    transfer (8 KiB partition lines, ~420GB/s across all 16 SDMA
    engines); tile 14 ships alone and tile 15 as quarters so the tail
    chain starts as soon as the last 128 KiB lands. All tiles are
    SBUF-resident, so nothing ever gates the stream.
  - constants (w pre-transposed [128, 16*13] fp16 + bias) ride the scalar
    HWDGE ring (never the gpsimd SWDGE: its descriptor rings contend with
    SDMA engines 7/15 and straggle the stream).
  - PE: per chunk one fp16 matmul, N=13, accumulated over the 16 chunks
    of a tile in PSUM (LDW ~35-90ns/chunk; far under the stream pace).
  - DVE: per-tile bias-add fold (PSUM read) + the logic-layer tail per
    half-batch; the mid-kernel half puts elementwise products on gpsimd,
    the final latency-exposed half runs entirely on DVE.
  - outputs: one contiguous [128, 40] block per half, emitted after the
    main loop on the sync ring (they drain behind the x backlog; nothing
    downstream reads them).

History: 95.1us (f16x3 on-device-transpose baseline) -> ~75us (f16x1 +
engine re-assignment + scheduling) -> host-pretransposed fp16 stream.
"""

import numpy as np

import concourse.bacc as bacc
import concourse.mybir as mybir
import concourse.tile as tile
from concourse.bass_utils import run_bass_kernel_spmd

F32 = mybir.dt.float32
F16 = mybir.dt.float16
AX = mybir.AxisListType
ADD = mybir.AluOpType.add

MODE = "f16host"

N_CORES = 8
B_FULL, H = 16384, 2048
B = B_FULL // N_CORES  # rows per core
P = 128
NT = B // P            # batch tiles per core
NCH = H // P           # contraction chunks
J = 13                 # 4 + 4 + 5 logits
NTAILS = 2
NTQ = NT // NTAILS     # tiles per tail half


def _build_program(mode):
    assert mode == "f16host"
    nc = bacc.Bacc("TRN2", target_bir_lowering=False, debug=False,
                   num_devices=N_CORES)
    # xt[pair][hh][u][c][bb] = x[(2*pair+u)*128+bb, c*128+hh] in fp16
    # (host pre-packed): two batch tiles share a DRAM row, so the bulk
    # stream moves 8 KiB partition lines in 1 MiB transfers.
    xt_d = nc.dram_tensor("xt", [NT // 2, P, 2 * NCH * P], F16,
                          kind="ExternalInput")
    w_d = nc.dram_tensor("w", [P, NCH * J], F16, kind="ExternalInput")
    b_d = nc.dram_tensor("b", [P, J], F32, kind="ExternalInput")
    y_d = nc.dram_tensor("y", [P, NT * 5], F32, kind="ExternalOutput")

    with tile.TileContext(nc) as tc:
        with (
            tc.tile_pool(name="const", bufs=1) as cpool,
            tc.tile_pool(name="xts", bufs=1) as xts_pool,
            tc.tile_pool(name="xtsq", bufs=1) as xtsq_pool,
            tc.tile_pool(name="acc", bufs=4, space="PSUM") as acc_pool,
            tc.tile_pool(name="work", bufs=1) as wpool,
            tc.tile_pool(name="tailp", bufs=2) as tpool,
        ):
            w_sb = cpool.tile([P, NCH, J], F16)
            b_sb = cpool.tile([P, J], F32)
            # constants first on the scalar ring: issued ~7.2us, land ~9us,
            # first accum needs w ~10.5us
            nc.scalar.dma_start(
                w_sb[:].rearrange("p c j -> p (c j)"), w_d.ap())
            nc.scalar.dma_start(b_sb[:], b_d.ap())

            all_st = [wpool.tile([P, NTQ, J], F32, tag=f"st{g}",
                                 name=f"all_st{g}")
                      for g in range(NTAILS)]
            out_tiles = [wpool.tile([P, NTQ, 5], F32, tag=f"ot{g}",
                                    name=f"out_sb{g}")
                         for g in range(NTAILS)]
            out_dmas = []

            def tail(g):
                # half 0 runs mid-kernel: elementwise products on the idle
                # gpsimd so DVE stays light (free-dim reductions +
                # reciprocal must stay on DVE). The final half is
                # latency-exposed: all-DVE avoids cross-engine hops.
                ew = nc.vector if g == NTAILS - 1 else nc.gpsimd
                st = all_st[g][:]
                e_all = tpool.tile([P, NTQ, J], F32, tag="e_all")
                nc.scalar.activation(e_all[:], st,
                                     mybir.ActivationFunctionType.Exp)
                EG = e_all[:, :, 0:4]
                EP = e_all[:, :, 4:8]
                EA = e_all[:, :, 8:13]

                tmp4 = tpool.tile([P, NTQ, 4], F32, tag="tmp4")
                ew.tensor_mul(tmp4[:], EP, EG)
                tmp2 = tpool.tile([P, NTQ, 2], F32, tag="tmp2")
                ew.tensor_mul(tmp2[:], e_all[:, :, 4:8:2],
                              e_all[:, :, 1:4:2])
                tmp2b = tpool.tile([P, NTQ, 2], F32, tag="tmp2b")
                ew.tensor_mul(tmp2b[:], e_all[:, :, 5:8:2],
                              e_all[:, :, 0:3:2])

                sg = tpool.tile([P, NTQ], F32, tag="sg")
                nc.vector.tensor_reduce(sg[:], EG, axis=AX.X, op=ADD)
                sp = tpool.tile([P, NTQ], F32, tag="sp")
                nc.vector.tensor_reduce(sp[:], EP, axis=AX.X, op=ADD)
                u3 = tpool.tile([P, NTQ, 3], F32, tag="u3")
                nc.vector.tensor_reduce(u3[:, :, 0], tmp4[:], axis=AX.X,
                                        op=ADD)
                nc.vector.tensor_reduce(u3[:, :, 1], tmp2[:], axis=AX.X,
                                        op=ADD)
                nc.vector.tensor_reduce(u3[:, :, 2], tmp2b[:], axis=AX.X,
                                        op=ADD)

                ss = tpool.tile([P, NTQ], F32, tag="ss")
                ew.tensor_mul(ss[:], sp[:], sg[:])
                V = tpool.tile([P, NTQ, 5], F32, tag="V")
                ew.tensor_sub(V[:, :, 0:3],
                              ss[:].broadcast_to([P, NTQ, 3]), u3[:])
                ew.tensor_copy(V[:, :, 3:5],
                               ss[:].broadcast_to([P, NTQ, 2]))
                tj = tpool.tile([P, NTQ, 5], F32, tag="tj")
                ew.tensor_mul(tj[:], EA, V[:])

                s5 = tpool.tile([P, NTQ], F32, tag="s5")
                nc.vector.tensor_reduce(s5[:], tj[:], axis=AX.X, op=ADD)
                r5 = tpool.tile([P, NTQ], F32, tag="r5")
                nc.vector.reciprocal(r5[:], s5[:])

                out_sb = out_tiles[g]
                ew.tensor_mul(out_sb[:], tj[:],
                              r5[:].broadcast_to([P, NTQ, 5]))
                out_dmas.append((g, out_sb))

            def fold(t, acc):
                # PSUM->SBUF logits stage with the bias add fused, on DVE
                nc.vector.tensor_add(all_st[t // NTQ][:, t % NTQ, :],
                                     acc[:], b_sb[:])
                # defer the mid-kernel tail a couple of tiles so its sems
                # never gate anything on the still-busy engines
                if t == NTQ + 1:
                    tail(0)
                elif t == NT - 1:
                    tail(1)

            # Stream plan: pairs 0..6 as full 1 MiB transfers (8 KiB
            # lines, fewest issues -> deepest queue during the ramp);
            # tile 14 as a single, tile 15 as quarters so the tail chain
            # starts as soon as the last 128 KiB lands. All tiles stay
            # SBUF-resident (no pool-recycle gating of the stream).
            CP = NCH * P                  # xt columns per tile
            tile_src = {}

            # single ring for the whole stream: a dual-ring split was
            # measured slower (the rings' packets interleave per SDMA
            # engine and break HBM access locality).
            for pr in range(NT // 2 - 1):
                xpt = xts_pool.tile([P, 2 * CP], F16, tag=f"pr{pr}",
                                    name=f"pr{pr}")
                nc.sync.dma_start(xpt[:], xt_d.ap()[pr, :, :])
                tile_src[2 * pr] = (xpt, 0)
                tile_src[2 * pr + 1] = (xpt, NCH)
            x14 = xts_pool.tile([P, CP], F16, tag="x14", name="x14")
            nc.sync.dma_start(x14[:], xt_d.ap()[NT // 2 - 1, :, 0:CP])
            tile_src[NT - 2] = (x14, 0)
            for q in range(4):
                CW = CP // 4
                xq15 = xtsq_pool.tile([P, CW], F16, tag=f"q{q}",
                                      name=f"x15_{q}")
                nc.sync.dma_start(
                    xq15[:],
                    xt_d.ap()[NT // 2 - 1, :, CP + q * CW:CP + (q + 1) * CW])
                tile_src[(NT - 1, q)] = (xq15, 0)

            for t in range(NT):
                acc = acc_pool.tile([P, J], F32)
                for c in range(NCH):
                    if t == NT - 1:
                        xq, coff = tile_src[(t, c // (NCH // 4))]
                        k = c % (NCH // 4)
                    else:
                        xq, coff = tile_src[t]
                        k = coff + c
                    nc.tensor.matmul(acc[:], xq[:, k * P:(k + 1) * P],
                                     w_sb[:, c, :],
                                     start=c == 0, stop=c == NCH - 1,
                                     skip_group_check=True)
                fold(t, acc)

            # output DMAs last: the sync engine has finished its x issues,
            # and the ring drains them after the x backlog.
            for g, out_sb in out_dmas:
                nc.sync.dma_start(
                    y_d.ap()[:, g * NTQ * 5:(g + 1) * NTQ * 5],
                    out_sb[:].rearrange("p t j -> p (t j)"))

    nc.compile()
    return nc


_NC_CACHE = {}


def _get_program(mode=MODE):
    if mode not in _NC_CACHE:
        _NC_CACHE[mode] = _build_program(mode)
    return _NC_CACHE[mode]


def _prep_in_maps(x, Wg, bg, Wp, bp, Wa, ba, mode=MODE):
    x = np.asarray(x, dtype=np.float32)
    W = np.concatenate([np.asarray(Wg), np.asarray(Wp), np.asarray(Wa)],
                       axis=1).astype(np.float32)
    bvec = np.concatenate([np.asarray(bg), np.asarray(bp), np.asarray(ba)]
                          ).astype(np.float32).reshape(1, J)
    # [h, j] -> [p, c*J+j] with h = c*128 + p (contiguous device load)
    w_dev = np.ascontiguousarray(
        W.astype(np.float16).reshape(NCH, P, J).transpose(1, 0, 2)
    ).reshape(P, NCH * J)
    b_dev = np.ascontiguousarray(np.broadcast_to(bvec, (P, J)),
                                 dtype=np.float32)
    # host pre-pack: xt[pair][hh][u][c][bb] = x[(2*pair+u)*128+bb, c*128+hh]
    x16 = x.astype(np.float16)
    in_maps = []
    for i in range(N_CORES):
        xc = x16[i * B:(i + 1) * B].reshape(NT // 2, 2, P, NCH, P)
        xt = np.ascontiguousarray(xc.transpose(0, 4, 1, 3, 2)).reshape(
            NT // 2, P, 2 * NCH * P)
        in_maps.append({
            "xt": xt,
            "w": w_dev,
            "b": b_dev,
        })
    return in_maps


def kernel(x, Wg, bg, Wp, bp, Wa, ba):
    in_maps = _prep_in_maps(x, Wg, bg, Wp, bp, Wa, ba)
    nc = _get_program()
    res = run_bass_kernel_spmd(nc, in_maps, core_ids=list(range(N_CORES)))
    outs = []
    for i in range(N_CORES):
        y = res.results[i]["y"]  # [P, NT*5], row b = t*P + p at [p, t*5+j]
        outs.append(y.reshape(P, NT, 5).transpose(1, 0, 2).reshape(B, 5))
    return np.concatenate(outs, axis=0)
